# revision 1
# baseline (speedup 1.0000x reference)
"""Trainium2 Bass kernel for nn_PlanNotesProjection.

Math (per batch b):
  own_f   = ownership[b].astype(f32)             # (K=32, S=4096)
  summed  = own_f @ emb[b]                       # (K, H=2048)
  counts  = clip(own_f.sum(-1), min=1)           # (K,)
  pooled  = summed / counts[:, None]
  proj    = pooled @ W + bias                    # (K, D=1024)
  out[b]  = LayerNorm(proj) * gamma + beta       # eps=1e-5

Structure: h-major. emb, ownership and W stream in fp8 e3m4 (4
mantissa bits — ~1.2% RMS quantization on N(0,1)-scaled data; 0/1
ownership is exact; W is host-prescaled by 64 into e3m4's normal
range, the scale absorbed via LayerNorm scale invariance; PSUM
accumulation stays fp32; the projection matmul runs mixed-dtype with
a bf16 stationary and fp8 moving operand — measured end-to-end rel
err 1.884e-2 against the f32 reference, under the 2e-2 gate). The host
pre-swizzles emb so each h-tile (128 columns of H) is one contiguous
[128, 4KB] region, streamed as two halves on the two HWDGE rings
(SP ring via nc.sync, Activation ring via nc.scalar) so both hardware
dynamic queues stream concurrently at a combined ~340 GB/s. Pooling
for h-tile h accumulates sumT_h[m, k] = sum_s emb[s, h*128+m] *
own[k, s] over the 32 S-chunks into a PSUM bank; the two projection
matmuls for tile h (contraction over H on partitions) are emitted one
tile later (software pipeline) so their LDWEIGHTS wait on the DVE
stage copy never bubbles the in-order PE queue.

The PSUM->SBUF stage copy runs on the Vector engine so the Scalar
engine's queue holds only DMA issues — a copy there would block the
next h-tile's emb DMA issue behind pooling completion.

When bias == 0 (the staged problem), LayerNorm's scale invariance
removes the per-element counts division: LN(v/c) = (v - mu_v)/
sqrt(var_v + eps*c^2), so only the rsqrt bias needs the per-k counts
correction eps_k = LN_EPS * clip(counts,1)^2 — the counts matmuls ride
in PE slack and the [K,D]-sized division disappears. kernel() checks
the actual inputs and rebuilds the general variant if bias/gamma/beta
are ever non-trivial.

Sharding: data-parallel over B across 8 cores (one batch per core);
W/b/gamma/beta replicated. Host swizzles make every device DMA fully
contiguous per partition:
  embP[p, (h*SC + c)*128 + j] = emb[c*128+p, h*128+j]   (fp8 e3m4)
  ownP[p, c*K + k]            = own[k, c*128+p]          (fp8 e3m4)
  wP[p, h*D + d]              = 64 * W[h*128+p, d]       (fp8 e3m4)
"""

import sys
from contextlib import ExitStack

import numpy as np

sys.path.insert(0, "/opt/trn_rl_repo")

import ml_dtypes

BF16 = ml_dtypes.bfloat16
FP8E3 = ml_dtypes.float8_e3m4

B, K, S, H, D = 8, 32, 4096, 2048, 1024
LN_EPS = 1e-5
P = 128
SC = S // P    # 32 contraction chunks (S on partitions)
HC = H // P    # 16 h-tiles
DJ = D // 512  # 2 psum column tiles for projection

TRACE = False
TRACE_TMPDIR = None
LAST_RESULT = None
_NC = None
_NC_KEY = None


def _prep_emb(emb_b):
    # (S, H) f32 -> (P, HC*SC*128) fp8 e3m4 with
    # embP[p, (h*SC+c)*128+j] = emb[c*128+p, h*128+j]
    # e3m4 (4 mantissa bits, normals down to 2^-2) quantizes N(0,1) data at
    # ~1.2% RMS; randn max |x| ~5.7 is far below the 15.5 format max.
    return np.ascontiguousarray(
        emb_b.astype(FP8E3).reshape(SC, P, HC, P).transpose(1, 2, 0, 3)
        .reshape(P, HC * SC * P))


def _prep_own(own_b):
    # (K, S) bool -> (P, SC*K) fp8 with ownP[p, c*K+k] = own[k, c*128+p]
    return np.ascontiguousarray(
        own_b.T.astype(FP8E3).reshape(SC, P, K).transpose(1, 0, 2).reshape(P, SC * K))


W_SCALE = 64.0  # lifts W ~N(0, 1/sqrt(2048)) into e3m4's normal range


def _prep_w(wmat):
    # (H, D) f32 -> (P, HC*D) fp8 e3m4, scaled by W_SCALE so the values
    # (sigma ~0.022, max ~0.12) land in e3m4 normals (2^-2..15.5): max
    # |W|*64 ~ 7.5 << 15.5, quantization ~1.2% RMS. The global scale is
    # absorbed by LayerNorm scale invariance (eps_k picks up W_SCALE^2).
    return np.ascontiguousarray(
        (wmat * W_SCALE).astype(FP8E3).reshape(HC, P, D).transpose(1, 0, 2)
        .reshape(P, HC * D))


def _build_nc(repeats=1, has_bias=False, has_gamma=False, has_beta=False):
    # repeats>1 unrolls the whole compute body (including DMAs) multiple
    # times in one NEFF; used by test.py to measure marginal per-iteration
    # HW time, cancelling host dispatch overhead. Grading uses repeats=1.
    import concourse.bass as bass
    import concourse.tile as tile
    from concourse import mybir
    from concourse.bacc import Bacc

    FP32 = mybir.dt.float32
    BF = mybir.dt.bfloat16
    F8 = mybir.dt.float8e3

    # Bacc (not plain Bass): its finalize() runs the legalization passes
    # (move_matmul_waits_to_ldweights, generate_event_semaphores) that split
    # multi-semaphore waits — TRN2 TPB instructions carry at most one.
    nc = Bacc("TRN2", target_bir_lowering=False)
    embP = nc.declare_dram_parameter("embP", [P, HC * SC * P], F8, False)
    ownP = nc.declare_dram_parameter("ownP", [P, SC * K], F8, False)
    wP = nc.declare_dram_parameter("wP", [P, HC * D], F8, False)
    bvec = nc.declare_dram_parameter("bvec", [D], FP32, False)
    gamma = nc.declare_dram_parameter("gamma", [D], FP32, False)
    beta = nc.declare_dram_parameter("beta", [D], FP32, False)
    out = nc.declare_dram_parameter("out", [K, D], FP32, True)

    with ExitStack() as ctx:
        tc = ctx.enter_context(tile.TileContext(nc))

        own_pool = ctx.enter_context(tc.tile_pool(name="own", bufs=1))
        w_pool = ctx.enter_context(tc.tile_pool(name="w", bufs=1))
        # Pool-recycled whole-h-tile buffers: consumer-pull pacing keeps the
        # DMA semaphore lanes unambiguous. Issuing the whole stream up-front
        # instead makes the tile framework coalesce late consumers onto one
        # issuer-drain event — a measured 41 us pooling stall.
        emb_pool = ctx.enter_context(tc.tile_pool(name="emb", bufs=12))
        ones_pool = ctx.enter_context(tc.tile_pool(name="ones", bufs=1))
        cnt_pool = ctx.enter_context(tc.tile_pool(name="cnt", bufs=4))
        st_pool = ctx.enter_context(tc.tile_pool(name="st", bufs=3))
        bc_pool = ctx.enter_context(tc.tile_pool(name="bc", bufs=1))
        x_pool = ctx.enter_context(tc.tile_pool(name="x", bufs=1))
        stats_pool = ctx.enter_context(tc.tile_pool(name="stats", bufs=1))
        # bufs=8: mv/negmu/vpe/rstd/nmr must live in DISTINCT slots — the
        # epilogue reads values from several generations back, which a
        # bufs=1 ring would have overwritten.
        mv_pool = ctx.enter_context(tc.tile_pool(name="mv", bufs=8))

        # Every PSUM allocation is rounded up to whole banks (bump_psum), so
        # each sumT ping-pong buffer owns a full bank: a start=True matmul's
        # whole-bank zero touches only its own accumulation group.
        psum_sum = ctx.enter_context(tc.tile_pool(name="psum_sum", bufs=2, space="PSUM"))
        psum_proj = ctx.enter_context(tc.tile_pool(name="psum_proj", bufs=1, space="PSUM"))
        psum_cnt = ctx.enter_context(tc.tile_pool(name="psum_cnt", bufs=1, space="PSUM"))

        HB = SC // 2  # half an h-tile's chunks per DMA

        def body():
            # own rides the SWDGE (gpsimd) queue: it frees the HWDGE rings'
            # first slots for emb and still lands well before pooling h=0.
            own_sb = own_pool.tile([P, SC, K], F8)
            nc.gpsimd.dma_start(out=own_sb[:], in_=ownP[:, :])

            # W quarter 0 on the scalar ring up-front; quarters 1..3
            # interleave with the emb stream so each ring carries ~2.1 MB
            # of W and the rings drain together.
            w_sb = w_pool.tile([P, HC, D], F8)
            nc.scalar.dma_start(w_sb[:, 0:4, :], wP[:, 0:4 * D])

            ones = ones_pool.tile([P, 1], F8)
            nc.vector.memset(ones[:], 1.0)

            def bcast(vec):
                t = bc_pool.tile([K, D], FP32, name=f"bc_{vec.name}")
                ap = vec[:]
                bc_ap = bass.AP(tensor=ap.tensor, offset=ap.offset, ap=[[0, K]] + list(ap.ap))
                nc.gpsimd.dma_start(out=t[:], in_=bc_ap)
                return t

            bias_bc = bcast(bvec) if has_bias else None
            gam_bc = bcast(gamma) if has_gamma else None
            bet_bc = bcast(beta) if has_beta else None

            cnt_ps = psum_cnt.tile([K, 1], FP32)
            proj_ps = [psum_proj.tile([K, 512], FP32, name=f"proj_ps{jj}") for jj in range(DJ)]
            pipe = []  # (h, st_sb) awaiting their proj matmuls

            def proj_step(hh, st, stop):
                for jj in range(DJ):
                    nc.tensor.matmul(proj_ps[jj][:], st[:],
                                     w_sb[:, hh, jj * 512:(jj + 1) * 512],
                                     start=(hh == 0), stop=stop)

            for h in range(HC):
                # Each h-tile streams as two halves on the two HWDGE rings
                # (SP via nc.sync, Activation via nc.scalar) so both
                # hardware dynamic queues run concurrently; consumer-pull
                # pacing through the 12-buffer pool keeps the 8 DMA
                # semaphore lanes unambiguous and the stream phase-stable.
                base = h * SC * P
                etA = emb_pool.tile([P, HB, P], F8)
                nc.sync.dma_start(etA[:], embP[:, base:base + HB * P])
                etB = emb_pool.tile([P, HB, P], F8)
                nc.scalar.dma_start(etB[:], embP[:, base + HB * P:base + SC * P])
                if h in (2, 5, 8):
                    q = {2: 1, 5: 2, 8: 3}[h]
                    weng = nc.scalar if q == 2 else nc.sync
                    weng.dma_start(w_sb[:, 4 * q:4 * (q + 1), :],
                                   wP[:, 4 * q * D:4 * (q + 1) * D])

                # Padded to 512 cols = 2KB = one full bank, so each ping-pong
                # buf owns its bank and start=True can't touch a neighbour.
                st_ps = psum_sum.tile([P, 512], FP32)
                for c in range(SC):
                    et = etA[:, c, :] if c < HB else etB[:, c - HB, :]
                    nc.tensor.matmul(st_ps[:, 0:K], et, own_sb[:, c, :],
                                     start=(c == 0), stop=(c == SC - 1))
                if h == 0:
                    # counts[k] = sum_s own[k, s] — fills the PE slack while
                    # the DVE stage copy for h=0 runs.
                    for c in range(SC):
                        nc.tensor.matmul(cnt_ps[:], own_sb[:, c, :], ones[:],
                                         start=(c == 0), stop=(c == SC - 1))
                # Stage copy on the Vector engine: the Scalar queue must stay
                # pure DMA issues, else the next h-tile's emb issue blocks
                # behind pooling completion.
                st_sb = st_pool.tile([P, K], BF)
                nc.vector.tensor_copy(out=st_sb[:], in_=st_ps[:, 0:K])
                # Software pipeline (depth 2): proj for tile h-2 is emitted
                # AFTER pool(h), so its LDWEIGHTS wait on the DVE copy(h-2)
                # resolves two pool-phases earlier instead of bubbling the
                # in-order PE queue (~730 ns per tile otherwise).
                pipe.append((h, st_sb))
                if len(pipe) > 1:
                    hh, st = pipe.pop(0)
                    proj_step(hh, st, stop=False)
            while pipe:
                hh, st = pipe.pop(0)
                proj_step(hh, st, stop=(hh == HC - 1))

            cnt_sb = cnt_pool.tile([K, 1], FP32)
            nc.vector.tensor_scalar_max(out=cnt_sb[:], in0=cnt_ps[:], scalar1=1.0)

            if has_bias:
                # General path: pooled = summed/counts must be materialized
                # before the bias add; LayerNorm uses the plain eps. proj_ps
                # carries the W_SCALE factor, so fold it into the divisor.
                cnt64 = cnt_pool.tile([K, 1], FP32)
                nc.vector.tensor_scalar_mul(out=cnt64[:], in0=cnt_sb[:], scalar1=W_SCALE)
                inv_sb = cnt_pool.tile([K, 1], FP32)
                nc.vector.reciprocal(out=inv_sb[:], in_=cnt64[:])
                eps_k = cnt_pool.tile([K, 1], FP32)
                nc.vector.memset(eps_k[:], LN_EPS)
                x = x_pool.tile([K, D], FP32)
                for jj in range(DJ):
                    nc.vector.tensor_scalar_mul(
                        out=x[:, jj * 512:(jj + 1) * 512], in0=proj_ps[jj][:], scalar1=inv_sb[:],
                    )
                nc.vector.tensor_add(out=x[:], in0=x[:], in1=bias_bc[:])
                src = [x[:, jj * 512:(jj + 1) * 512] for jj in range(DJ)]
            else:
                # LN scale invariance: normalize raw W_SCALE*summed@W
                # directly; only the rsqrt bias needs the per-row scale:
                # eps_k = LN_EPS * (counts * W_SCALE)^2.
                cnt2 = cnt_pool.tile([K, 1], FP32)
                nc.vector.tensor_mul(out=cnt2[:], in0=cnt_sb[:], in1=cnt_sb[:])
                eps_k = cnt_pool.tile([K, 1], FP32)
                nc.vector.tensor_scalar_mul(out=eps_k[:], in0=cnt2[:],
                                            scalar1=LN_EPS * W_SCALE * W_SCALE)
                src = [proj_ps[jj][:] for jj in range(DJ)]

            stats = stats_pool.tile([K, DJ, nc.vector.BN_STATS_DIM], FP32)
            for g in range(DJ):
                nc.vector.bn_stats(out=stats[:, g, :], in_=src[g])
            mv = mv_pool.tile([K, nc.vector.BN_AGGR_DIM], FP32)
            nc.vector.bn_aggr(out=mv[:], in_=stats[:])
            # rstd = 1/sqrt(var + eps_k). Keep Sqrt as the ONLY scalar-engine
            # activation function: a second func would trigger a ~1.3 us
            # ACT_TABLE swap mid-kernel.
            rstd = mv_pool.tile([K, 1], FP32)
            nc.scalar.activation(
                out=rstd[:], in_=mv[:, 1:2],
                func=mybir.ActivationFunctionType.Sqrt, bias=eps_k[:], scale=1.0, alpha=0.0,
            )
            nc.vector.reciprocal(out=rstd[:], in_=rstd[:])
            outt = x_pool.tile([K, D], FP32)
            for jj in range(DJ):
                half = outt[:, jj * 512:(jj + 1) * 512]
                nc.vector.tensor_scalar(
                    out=half, in0=src[jj], scalar1=mv[:, 0:1], scalar2=rstd[:],
                    op0=mybir.AluOpType.subtract, op1=mybir.AluOpType.mult,
                )
                if has_gamma:
                    nc.vector.tensor_mul(out=half, in0=half, in1=gam_bc[:, jj * 512:(jj + 1) * 512])
                if has_beta:
                    nc.vector.tensor_add(out=half, in0=half, in1=bet_bc[:, jj * 512:(jj + 1) * 512])
                oeng = nc.sync if jj == 0 else nc.scalar
                oeng.dma_start(out[:, jj * 512:(jj + 1) * 512], half)

        for _ in range(repeats):
            body()

    nc.finalize()
    return nc


def kernel(**inputs: np.ndarray) -> np.ndarray:
    global _NC, _NC_KEY, LAST_RESULT
    from concourse.bass_utils import run_bass_kernel_spmd

    emb = np.asarray(inputs["plan_embeddings"], dtype=np.float32)
    own = np.asarray(inputs["ownership"])
    wmat = np.ascontiguousarray(np.asarray(inputs["W"], dtype=np.float32))
    bv = np.ascontiguousarray(np.asarray(inputs["b"], dtype=np.float32))
    ga = np.ascontiguousarray(np.asarray(inputs["gamma"], dtype=np.float32))
    be = np.ascontiguousarray(np.asarray(inputs["beta"], dtype=np.float32))

    key = (bool(np.any(bv != 0.0)), bool(np.any(ga != 1.0)), bool(np.any(be != 0.0)))
    if _NC is None or _NC_KEY != key:
        _NC = _build_nc(has_bias=key[0], has_gamma=key[1], has_beta=key[2])
        _NC_KEY = key

    wP = _prep_w(wmat)
    in_maps = []
    for i in range(B):
        in_maps.append({
            "embP": _prep_emb(emb[i]),
            "ownP": _prep_own(own[i]),
            "wP": wP,
            "bvec": bv,
            "gamma": ga,
            "beta": be,
        })
    res = run_bass_kernel_spmd(_NC, in_maps, core_ids=list(range(B)), trace=TRACE,
                               tmpdir=TRACE_TMPDIR)
    LAST_RESULT = res
    return np.stack([np.asarray(res.results[i]["out"]) for i in range(B)], axis=0).astype(np.float32)



# revision 2
# speedup vs baseline: 2.5864x; 2.5864x over previous
"""Trainium2 Bass kernel for nn_PlanNotesProjection — v2 (tail-optimized).

Math (per batch b):
  own_f   = ownership[b].astype(f32)             # (K=32, S=4096)
  summed  = own_f @ emb[b]                       # (K, H=2048)
  counts  = clip(own_f.sum(-1), min=1)           # (K,)
  pooled  = summed / counts[:, None]
  proj    = pooled @ W + bias                    # (K, D=1024)
  out[b]  = LayerNorm(proj) * gamma + beta       # eps=1e-5

v2 changes vs v1 (all aimed at the post-stream tail, which the timeline
sim showed was 8.25us of the 39.7us total):
  1. Row-mean via PE: mean_k = sum_d proj_kd = pooled @ wsum where
     wsum[p] = sum_d Wq[p,d] is host-precomputed from the quantized W.
     One extra 1-column matmul per h-tile rides in PE slack; the mean is
     ready before the last projection finishes, so the LN epilogue needs
     only a sum-of-squares pass (no bn_stats serial chain).
  2. Parallel sumsq: bank0 via ACT activation(Square, accum_out);
     bank1 via DVE bn_stats (a DVE op may read only one PSUM input, so
     sumsq1 = 512*(var1+mean1^2)) — the two legs run concurrently.
  3. Parallel normalize: bank0 via DVE tensor_scalar(sub mu, mult rstd);
     bank1 via ACT activation(Identity, scale=rstd, bias=-mu*rstd).
     Square/Sqrt/Identity all live in one ACT table (sqrt_and_friends)
     so no mid-kernel ACT table swap.
  4. Single [K,1024] store (one 625ns HWDGE desc-gen + one 650ns DGE
     launch instead of two serialized chains).
  5. Tail scheduling: proj(14) and proj(15) are emitted after pool(15)
     so proj(14) (whose operands are long ready) fills the PE while the
     DVE stage-copy of tile 15 round-trips; tile 14's halves swap rings
     so the slower Act-ring SEQ doesn't reorder the last arrivals.

Everything else (fp8 e3m4 streaming, host swizzles, two HWDGE rings,
depth-2 proj software pipeline, LN scale-invariance absorbing the
counts division and W_SCALE) is unchanged from v1 — see below.

Sharding: data-parallel over B across 8 cores (one batch per core);
W/b/gamma/beta replicated. Host swizzles make every device DMA fully
contiguous per partition:
  embP[p, (h*SC + c)*128 + j] = emb[c*128+p, h*128+j]   (fp8 e3m4)
  ownP[p, c*K + k]            = own[k, c*128+p]          (fp8 e3m4)
  wP[p, h*D + d]              = 64 * W[h*128+p, d]       (fp8 e3m4)
  wsumP[p, h]                 = sum_d fp8(64*W)[h*128+p, d]  (bf16)
"""

import sys
from contextlib import ExitStack

import numpy as np

sys.path.insert(0, "/opt/trn_rl_repo")

import ml_dtypes

BF16 = ml_dtypes.bfloat16
FP8E3 = ml_dtypes.float8_e3m4

B, K, S, H, D = 8, 32, 4096, 2048, 1024
LN_EPS = 1e-5
P = 128
SC = S // P    # 32 contraction chunks (S on partitions)
HC = H // P    # 16 h-tiles
DJ = D // 512  # 2 psum column tiles for projection
HB = SC // 2   # half an h-tile's chunks per DMA
LB = 4         # trailing chunks in the split final DMA

TRACE = False
TRACE_TMPDIR = None
LAST_RESULT = None
_NC = None
_NC_KEY = None


def _prep_emb(emb_b):
    # (S, H) f32 -> (P, HC*SC*128) fp8 e3m4 with
    # embP[p, (h*SC+c)*128+j] = emb[c*128+p, h*128+j]
    return np.ascontiguousarray(
        emb_b.astype(FP8E3).reshape(SC, P, HC, P).transpose(1, 2, 0, 3)
        .reshape(P, HC * SC * P))


def _prep_own(own_b):
    # (K, S) bool -> (P, SC*K) fp8 with ownP[p, c*K+k] = own[k, c*128+p]
    return np.ascontiguousarray(
        own_b.T.astype(FP8E3).reshape(SC, P, K).transpose(1, 0, 2).reshape(P, SC * K))


W_SCALE = 64.0  # lifts W ~N(0, 1/sqrt(2048)) into e3m4's normal range


def _prep_w(wmat):
    # (H, D) f32 -> (P, HC*D) fp8 e3m4, scaled by W_SCALE; the global scale
    # is absorbed by LayerNorm scale invariance (eps_k picks up W_SCALE^2).
    return np.ascontiguousarray(
        (wmat * W_SCALE).astype(FP8E3).reshape(HC, P, D).transpose(1, 0, 2)
        .reshape(P, HC * D))


def _prep_wsum(wmat):
    # Row sums of the QUANTIZED scaled W, f32-accumulated then bf16:
    # wsumP[p, h] = sum_d fp8(64*W)[h*128+p, d].  Feeds the PE row-mean
    # matmul so the device mean matches sum_d of the actual proj values.
    wq = (wmat * W_SCALE).astype(FP8E3).astype(np.float32)  # (H, D)
    return np.ascontiguousarray(
        wq.sum(axis=1).reshape(HC, P).T.astype(BF16))  # (P, HC)


def _build_nc(repeats=1, has_bias=False, has_gamma=False, has_beta=False):
    import concourse.bass as bass
    import concourse.tile as tile
    from concourse import mybir
    from concourse.bacc import Bacc

    FP32 = mybir.dt.float32
    BF = mybir.dt.bfloat16
    F8 = mybir.dt.float8e3

    nc = Bacc("TRN2", target_bir_lowering=False)
    embP = nc.declare_dram_parameter("embP", [P, HC * SC * P], F8, False)
    ownP = nc.declare_dram_parameter("ownP", [P, SC * K], F8, False)
    wP = nc.declare_dram_parameter("wP", [P, HC * D], F8, False)
    wsumP = nc.declare_dram_parameter("wsumP", [P, HC], BF, False)
    bvec = nc.declare_dram_parameter("bvec", [D], FP32, False)
    gamma = nc.declare_dram_parameter("gamma", [D], FP32, False)
    beta = nc.declare_dram_parameter("beta", [D], FP32, False)
    out = nc.declare_dram_parameter("out", [K, D], FP32, True)

    with ExitStack() as ctx:
        tc = ctx.enter_context(tile.TileContext(nc))

        own_pool = ctx.enter_context(tc.tile_pool(name="own", bufs=1))
        w_pool = ctx.enter_context(tc.tile_pool(name="w", bufs=1))
        wsum_pool = ctx.enter_context(tc.tile_pool(name="wsum", bufs=1))
        # Pool-recycled whole-h-tile buffers: consumer-pull pacing keeps the
        # DMA semaphore lanes unambiguous.
        emb_pool = ctx.enter_context(tc.tile_pool(name="emb", bufs=16))
        ones_pool = ctx.enter_context(tc.tile_pool(name="ones", bufs=2))
        cnt_pool = ctx.enter_context(tc.tile_pool(name="cnt", bufs=4))
        st_pool = ctx.enter_context(tc.tile_pool(name="st", bufs=3))
        bc_pool = ctx.enter_context(tc.tile_pool(name="bc", bufs=1))
        x_pool = ctx.enter_context(tc.tile_pool(name="x", bufs=3))
        stats_pool = ctx.enter_context(tc.tile_pool(name="stats", bufs=1))
        # distinct slots for the many small [K,1] epilogue vectors
        mv_pool = ctx.enter_context(tc.tile_pool(name="mv", bufs=10))

        psum_sum = ctx.enter_context(tc.tile_pool(name="psum_sum", bufs=2, space="PSUM"))
        psum_proj = ctx.enter_context(tc.tile_pool(name="psum_proj", bufs=1, space="PSUM"))
        psum_cnt = ctx.enter_context(tc.tile_pool(name="psum_cnt", bufs=1, space="PSUM"))
        psum_mean = ctx.enter_context(tc.tile_pool(name="psum_mean", bufs=1, space="PSUM"))

        def body():
            # own + wsum ride the SWDGE (gpsimd) queue: frees the HWDGE
            # rings' first slots for emb; both land well before their use.
            own_sb = own_pool.tile([P, SC, K], F8)
            nc.gpsimd.dma_start(out=own_sb[:], in_=ownP[:, :])
            wsum_sb = wsum_pool.tile([P, HC], BF)
            nc.gpsimd.dma_start(out=wsum_sb[:], in_=wsumP[:, :])

            # W quarter 0 on the scalar ring up-front; quarters 1..3
            # interleave with the emb stream.
            w_sb = w_pool.tile([P, HC, D], F8)
            nc.scalar.dma_start(w_sb[:, 0:4, :], wP[:, 0:4 * D])

            ones = ones_pool.tile([P, 1], F8)
            nc.vector.memset(ones[:], 1.0)

            # Dummy Sqrt: the act-table pass greedily picks the first table
            # containing the FIRST activation func seen; Sqrt's first match
            # (set 3, sqrt_and_others) also contains Square and Identity, so
            # this pins ONE table load, hoisted to kernel start and hidden
            # under the emb stream. Without it, Square first-matches set 0
            # and the Sqrt forces a 1283ns mid-epilogue table swap.
            dum = ones_pool.tile([1, 1], FP32)
            nc.vector.memset(dum[:], 1.0)
            nc.scalar.activation(out=dum[:], in_=dum[:],
                                 func=mybir.ActivationFunctionType.Sqrt,
                                 bias=0.0, scale=1.0, alpha=0.0)

            def bcast(vec):
                t = bc_pool.tile([K, D], FP32, name=f"bc_{vec.name}")
                ap = vec[:]
                bc_ap = bass.AP(tensor=ap.tensor, offset=ap.offset, ap=[[0, K]] + list(ap.ap))
                nc.gpsimd.dma_start(out=t[:], in_=bc_ap)
                return t

            bias_bc = bcast(bvec) if has_bias else None
            gam_bc = bcast(gamma) if has_gamma else None
            bet_bc = bcast(beta) if has_beta else None

            cnt_ps = psum_cnt.tile([K, 1], FP32)
            mean_ps = psum_mean.tile([K, 1], FP32)
            proj_ps = [psum_proj.tile([K, 512], FP32, name=f"proj_ps{jj}") for jj in range(DJ)]
            pipe = []  # (h, st_sb) awaiting their proj matmuls

            def proj_step(hh, st, stop):
                # mean first: its stop fires ~2 matmuls before the banks',
                # giving the DVE time to fold mu into bias_k during proj.
                nc.tensor.matmul(mean_ps[:], st[:], wsum_sb[:, hh:hh + 1],
                                 start=(hh == 0), stop=stop)
                for jj in range(DJ):
                    nc.tensor.matmul(proj_ps[jj][:], st[:],
                                     w_sb[:, hh, jj * 512:(jj + 1) * 512],
                                     start=(hh == 0), stop=stop)

            for h in range(HC):
                # Each h-tile streams as two halves on the two HWDGE rings
                # (SP via nc.sync, Activation ring via nc.scalar). The LAST
                # FOUR pieces (etB14, etA15, etB15x, etB15y) all ride ring A:
                # a single ring's pieces arrive strictly in issue order, so
                # the pooling emission order is guaranteed to match data
                # arrival at the stream tail (cross-ring arbitration was
                # measured inverting etB14/etA15, stalling the in-order PE
                # queue ~700ns). Tile 14's A-half compensates on ring B. The
                # final piece is only 4 chunks so just 52ns of pooling trails
                # the last DMA's 900ns completion-semaphore propagation.
                base = h * SC * P
                last = h == HC - 1
                engA = nc.scalar if h == HC - 2 else nc.sync
                etA = emb_pool.tile([P, HB, P], F8)
                engA.dma_start(etA[:], embP[:, base:base + HB * P])
                if not last:
                    engB = nc.sync if h == HC - 2 else nc.scalar
                    etB = emb_pool.tile([P, HB, P], F8)
                    engB.dma_start(etB[:], embP[:, base + HB * P:base + SC * P])
                else:
                    etB = emb_pool.tile([P, HB, P], F8)
                    nc.sync.dma_start(etB[:], embP[:, base + HB * P:base + SC * P])
                if h in (2, 5, 8):
                    q = {2: 1, 5: 2, 8: 3}[h]
                    weng = nc.scalar if q == 2 else nc.sync
                    weng.dma_start(w_sb[:, 4 * q:4 * (q + 1), :],
                                   wP[:, 4 * q * D:4 * (q + 1) * D])

                st_ps = psum_sum.tile([P, 512], FP32)
                for c in range(HB):
                    nc.tensor.matmul(st_ps[:, 0:K], etA[:, c, :], own_sb[:, c, :],
                                     start=(c == 0), stop=False)
                for c in range(HB, SC):
                    nc.tensor.matmul(st_ps[:, 0:K], etB[:, c - HB, :], own_sb[:, c, :],
                                     start=False, stop=(c == SC - 1))
                if h == 0:
                    # counts[k] = sum_s own[k, s] — fills PE slack.
                    for c in range(SC):
                        nc.tensor.matmul(cnt_ps[:], own_sb[:, c, :], ones[:],
                                         start=(c == 0), stop=(c == SC - 1))
                # Stage copy on the Vector engine (Scalar queue stays pure
                # DMA issues mid-stream).
                st_sb = st_pool.tile([P, K], BF)
                nc.vector.tensor_copy(out=st_sb[:], in_=st_ps[:, 0:K])
                pipe.append((h, st_sb))
                if len(pipe) > 1 and not last:
                    hh, st = pipe.pop(0)
                    proj_step(hh, st, stop=False)
            # pipe holds (14, 15): proj(14) lands right after pool(15)'s
            # last chunk (its operands are long ready) and overlaps the DVE
            # stage copy of tile 15; proj(15) follows.
            hh, st = pipe.pop(0)
            proj_step(hh, st, stop=False)
            hh, st = pipe.pop(0)
            proj_step(hh, st, stop=True)

            cnt_sb = cnt_pool.tile([K, 1], FP32)
            nc.vector.tensor_scalar_max(out=cnt_sb[:], in0=cnt_ps[:], scalar1=1.0)

            if has_bias:
                # General path: pooled = summed/counts materialized before
                # the bias add; plain-eps LayerNorm via bn_stats.
                cnt64 = cnt_pool.tile([K, 1], FP32)
                nc.vector.tensor_scalar_mul(out=cnt64[:], in0=cnt_sb[:], scalar1=W_SCALE)
                inv_sb = cnt_pool.tile([K, 1], FP32)
                nc.vector.reciprocal(out=inv_sb[:], in_=cnt64[:])
                eps_k = cnt_pool.tile([K, 1], FP32)
                nc.vector.memset(eps_k[:], LN_EPS)
                x = x_pool.tile([K, D], FP32)
                for jj in range(DJ):
                    nc.vector.tensor_scalar_mul(
                        out=x[:, jj * 512:(jj + 1) * 512], in0=proj_ps[jj][:], scalar1=inv_sb[:],
                    )
                nc.vector.tensor_add(out=x[:], in0=x[:], in1=bias_bc[:])
                src = [x[:, jj * 512:(jj + 1) * 512] for jj in range(DJ)]

                stats = stats_pool.tile([K, DJ, nc.vector.BN_STATS_DIM], FP32)
                for g in range(DJ):
                    nc.vector.bn_stats(out=stats[:, g, :], in_=src[g])
                mv = mv_pool.tile([K, nc.vector.BN_AGGR_DIM], FP32)
                nc.vector.bn_aggr(out=mv[:], in_=stats[:])
                rstd = mv_pool.tile([K, 1], FP32)
                nc.scalar.activation(
                    out=rstd[:], in_=mv[:, 1:2],
                    func=mybir.ActivationFunctionType.Sqrt, bias=eps_k[:], scale=1.0, alpha=0.0,
                )
                nc.vector.reciprocal(out=rstd[:], in_=rstd[:])
                outt = x_pool.tile([K, D], FP32)
                for jj in range(DJ):
                    half = outt[:, jj * 512:(jj + 1) * 512]
                    nc.vector.tensor_scalar(
                        out=half, in0=src[jj], scalar1=mv[:, 0:1], scalar2=rstd[:],
                        op0=mybir.AluOpType.subtract, op1=mybir.AluOpType.mult,
                    )
                    if has_gamma:
                        nc.vector.tensor_mul(out=half, in0=half, in1=gam_bc[:, jj * 512:(jj + 1) * 512])
                    if has_beta:
                        nc.vector.tensor_add(out=half, in0=half, in1=bet_bc[:, jj * 512:(jj + 1) * 512])
                    oeng = nc.sync if jj == 0 else nc.scalar
                    oeng.dma_start(out[:, jj * 512:(jj + 1) * 512], half)
                return

            # Fast path (bias==0, gamma==1, beta==0): LN scale invariance
            # normalizes the raw v = W_SCALE*summed@W directly.
            #   mu      = mean_ps / D          (mean_ps = sum_d v, from PE)
            #   var     = sumsq/D - mu^2
            #   rstd    = 1/sqrt(var + eps_k),  eps_k = LN_EPS*(c*W_SCALE)^2
            #   out     = (v - mu) * rstd
            cnt2 = cnt_pool.tile([K, 1], FP32)
            nc.vector.tensor_mul(out=cnt2[:], in0=cnt_sb[:], in1=cnt_sb[:])
            eps_k = cnt_pool.tile([K, 1], FP32)
            nc.vector.tensor_scalar_mul(out=eps_k[:], in0=cnt2[:],
                                        scalar1=LN_EPS * W_SCALE * W_SCALE)

            mu = mv_pool.tile([K, 1], FP32)
            nc.vector.tensor_scalar_mul(out=mu[:], in0=mean_ps[:], scalar1=1.0 / D)
            musq = mv_pool.tile([K, 1], FP32)
            nc.vector.tensor_mul(out=musq[:], in0=mu[:], in1=mu[:])
            # bias_k = eps_k - mu^2, folded into the Sqrt's bias so the
            # post-sumsq chain is just add -> sqrt -> recip -> negmurstd.
            bias_k = mv_pool.tile([K, 1], FP32)
            nc.vector.tensor_sub(out=bias_k[:], in0=eps_k[:], in1=musq[:])

            # bank0's accumulation stops ~213ns before bank1's, so the ACT
            # engine (slower leg: 612 + 187 accumulator read) takes bank0
            # and the DVE takes bank1 — both legs finish together. The DVE
            # leg uses bn_stats (a DVE op may read only ONE input from PSUM,
            # which rules out tensor_tensor_reduce(x,x)); bank1's sum of
            # squares is recovered as 512*(var1 + mean1^2).
            sumsq0 = mv_pool.tile([K, 1], FP32)
            scr0 = x_pool.tile([K, 512], FP32)
            nc.scalar.activation(
                out=scr0[:], in_=proj_ps[0][:],
                func=mybir.ActivationFunctionType.Square,
                bias=0.0, scale=1.0, alpha=0.0, accum_out=sumsq0[:],
            )
            stats1 = stats_pool.tile([K, 1, nc.vector.BN_STATS_DIM], FP32)
            nc.vector.bn_stats(out=stats1[:, 0, :], in_=proj_ps[1][:])
            mv1 = mv_pool.tile([K, nc.vector.BN_AGGR_DIM], FP32)
            nc.vector.bn_aggr(out=mv1[:], in_=stats1[:])
            m1sq = mv_pool.tile([K, 1], FP32)
            nc.vector.tensor_mul(out=m1sq[:], in0=mv1[:, 0:1], in1=mv1[:, 0:1])
            h1 = mv_pool.tile([K, 1], FP32)
            nc.vector.tensor_add(out=h1[:], in0=mv1[:, 1:2], in1=m1sq[:])
            # t = 0.5*h1 + (eps_k - mu^2); full sqrt arg = sumsq0/D + t is
            # folded into the Sqrt's scale/bias (sumsq0 is ACT-side SBUF).
            tk = mv_pool.tile([K, 1], FP32)
            nc.vector.tensor_scalar(
                out=tk[:], in0=h1[:], scalar1=0.5, scalar2=bias_k[:],
                op0=mybir.AluOpType.mult, op1=mybir.AluOpType.add,
            )
            rstd = mv_pool.tile([K, 1], FP32)
            nc.scalar.activation(
                out=rstd[:], in_=sumsq0[:],
                func=mybir.ActivationFunctionType.Sqrt,
                bias=tk[:], scale=1.0 / D, alpha=0.0,
            )
            nc.vector.reciprocal(out=rstd[:], in_=rstd[:])
            negmurstd = mv_pool.tile([K, 1], FP32)
            nc.vector.tensor_scalar(
                out=negmurstd[:], in0=mu[:], scalar1=rstd[:], scalar2=-1.0,
                op0=mybir.AluOpType.mult, op1=mybir.AluOpType.mult,
            )
            outt = x_pool.tile([K, D], FP32)
            nc.vector.tensor_scalar(
                out=outt[:, 0:512], in0=proj_ps[0][:], scalar1=mu[:], scalar2=rstd[:],
                op0=mybir.AluOpType.subtract, op1=mybir.AluOpType.mult,
            )
            nc.scalar.activation(
                out=outt[:, 512:1024], in_=proj_ps[1][:],
                func=mybir.ActivationFunctionType.Identity,
                bias=negmurstd[:], scale=rstd[:], alpha=0.0,
            )
            nc.sync.dma_start(out[:, :], outt[:])

        for _ in range(repeats):
            body()

    nc.finalize()
    return nc


def kernel(**inputs: np.ndarray) -> np.ndarray:
    global _NC, _NC_KEY, LAST_RESULT
    from concourse.bass_utils import run_bass_kernel_spmd

    emb = np.asarray(inputs["plan_embeddings"], dtype=np.float32)
    own = np.asarray(inputs["ownership"])
    wmat = np.ascontiguousarray(np.asarray(inputs["W"], dtype=np.float32))
    bv = np.ascontiguousarray(np.asarray(inputs["b"], dtype=np.float32))
    ga = np.ascontiguousarray(np.asarray(inputs["gamma"], dtype=np.float32))
    be = np.ascontiguousarray(np.asarray(inputs["beta"], dtype=np.float32))

    key = (bool(np.any(bv != 0.0)), bool(np.any(ga != 1.0)), bool(np.any(be != 0.0)))
    if _NC is None or _NC_KEY != key:
        _NC = _build_nc(has_bias=key[0], has_gamma=key[1], has_beta=key[2])
        _NC_KEY = key

    wP = _prep_w(wmat)
    wsumP = _prep_wsum(wmat)
    in_maps = []
    for i in range(B):
        in_maps.append({
            "embP": _prep_emb(emb[i]),
            "ownP": _prep_own(own[i]),
            "wP": wP,
            "wsumP": wsumP,
            "bvec": bv,
            "gamma": ga,
            "beta": be,
        })
    res = run_bass_kernel_spmd(_NC, in_maps, core_ids=list(range(B)), trace=TRACE,
                               tmpdir=TRACE_TMPDIR)
    LAST_RESULT = res
    return np.stack([np.asarray(res.results[i]["out"]) for i in range(B)], axis=0).astype(np.float32)


# revision 3
# speedup vs baseline: 2.9240x; 1.1305x over previous
"""Trainium2 Bass kernel for nn_PlanNotesProjection — v2 (tail-optimized).

Math (per batch b):
  own_f   = ownership[b].astype(f32)             # (K=32, S=4096)
  summed  = own_f @ emb[b]                       # (K, H=2048)
  counts  = clip(own_f.sum(-1), min=1)           # (K,)
  pooled  = summed / counts[:, None]
  proj    = pooled @ W + bias                    # (K, D=1024)
  out[b]  = LayerNorm(proj) * gamma + beta       # eps=1e-5

v2 changes vs v1 (all aimed at the post-stream tail, which the timeline
sim showed was 8.25us of the 39.7us total):
  1. Row-mean via PE: mean_k = sum_d proj_kd = pooled @ wsum where
     wsum[p] = sum_d Wq[p,d] is host-precomputed from the quantized W.
     One extra 1-column matmul per h-tile rides in PE slack; the mean is
     ready before the last projection finishes, so the LN epilogue needs
     only a sum-of-squares pass (no bn_stats serial chain).
  2. Parallel sumsq: bank0 via ACT activation(Square, accum_out);
     bank1 via DVE bn_stats (a DVE op may read only one PSUM input, so
     sumsq1 = 512*(var1+mean1^2)) — the two legs run concurrently.
  3. Parallel normalize: bank0 via DVE tensor_scalar(sub mu, mult rstd);
     bank1 via ACT activation(Identity, scale=rstd, bias=-mu*rstd).
     Square/Sqrt/Identity all live in one ACT table (sqrt_and_friends)
     so no mid-kernel ACT table swap.
  4. Single [K,1024] store (one 625ns HWDGE desc-gen + one 650ns DGE
     launch instead of two serialized chains).
  5. Tail scheduling: proj(14) and proj(15) are emitted after pool(15)
     so proj(14) (whose operands are long ready) fills the PE while the
     DVE stage-copy of tile 15 round-trips; tile 14's halves swap rings
     so the slower Act-ring SEQ doesn't reorder the last arrivals.

Everything else (fp8 e3m4 streaming, host swizzles, two HWDGE rings,
depth-2 proj software pipeline, LN scale-invariance absorbing the
counts division and W_SCALE) is unchanged from v1 — see below.

Sharding: data-parallel over B across 8 cores (one batch per core);
W/b/gamma/beta replicated. Host swizzles make every device DMA fully
contiguous per partition:
  embP[p, (h*SC + c)*128 + j] = emb[c*128+p, h*128+j]   (fp8 e3m4)
  ownP[p, c*K + k]            = own[k, c*128+p]          (fp8 e3m4)
  wP[p, h*D + d]              = 64 * W[h*128+p, d]       (fp8 e3m4)
  wsumP[p, h]                 = sum_d fp8(64*W)[h*128+p, d]  (bf16)
"""

import sys
from contextlib import ExitStack

import numpy as np

sys.path.insert(0, "/opt/trn_rl_repo")

import ml_dtypes

BF16 = ml_dtypes.bfloat16
FP8E3 = ml_dtypes.float8_e3m4

B, K, S, H, D = 8, 32, 4096, 2048, 1024
LN_EPS = 1e-5
P = 128
SC = S // P    # 32 contraction chunks (S on partitions)
HC = H // P    # 16 h-tiles
DJ = D // 512  # 2 psum column tiles for projection
HB = SC // 2   # half an h-tile's chunks per DMA
LB = 4         # trailing chunks in the split final DMA

TRACE = False
TRACE_TMPDIR = None
LAST_RESULT = None
_NC = None
_NC_KEY = None


def _prep_emb(emb_b):
    # (S, H) f32 -> (P, HC*SC*128) fp8 e3m4 with
    # embP[p, (h*SC+c)*128+j] = emb[c*128+p, h*128+j]
    return np.ascontiguousarray(
        emb_b.astype(FP8E3).reshape(SC, P, HC, P).transpose(1, 2, 0, 3)
        .reshape(P, HC * SC * P))


def _prep_own(own_b):
    # (K, S) bool -> (P, SC*K) fp8 with ownP[p, c*K+k] = own[k, c*128+p]
    return np.ascontiguousarray(
        own_b.T.astype(FP8E3).reshape(SC, P, K).transpose(1, 0, 2).reshape(P, SC * K))


W_SCALE = 64.0  # lifts W ~N(0, 1/sqrt(2048)) into e3m4's normal range


def _prep_w(wmat):
    # (H, D) f32 -> (P, HC*D) fp8 e3m4, scaled by W_SCALE; the global scale
    # is absorbed by LayerNorm scale invariance (eps_k picks up W_SCALE^2).
    return np.ascontiguousarray(
        (wmat * W_SCALE).astype(FP8E3).reshape(HC, P, D).transpose(1, 0, 2)
        .reshape(P, HC * D))


def _prep_wsum(wmat):
    # Row sums of the QUANTIZED scaled W, f32-accumulated then bf16:
    # wsumP[p, h] = sum_d fp8(64*W)[h*128+p, d].  Feeds the PE row-mean
    # matmul so the device mean matches sum_d of the actual proj values.
    wq = (wmat * W_SCALE).astype(FP8E3).astype(np.float32)  # (H, D)
    return np.ascontiguousarray(
        wq.sum(axis=1).reshape(HC, P).T.astype(BF16))  # (P, HC)


def _build_nc(repeats=1, has_bias=False, has_gamma=False, has_beta=False):
    import concourse.bass as bass
    import concourse.tile as tile
    from concourse import mybir
    from concourse.bacc import Bacc

    FP32 = mybir.dt.float32
    BF = mybir.dt.bfloat16
    F8 = mybir.dt.float8e3

    nc = Bacc("TRN2", target_bir_lowering=False)
    embP = nc.declare_dram_parameter("embP", [P, HC * SC * P], F8, False)
    ownP = nc.declare_dram_parameter("ownP", [P, SC * K], F8, False)
    wP = nc.declare_dram_parameter("wP", [P, HC * D], F8, False)
    wsumP = nc.declare_dram_parameter("wsumP", [P, HC], BF, False)
    bvec = nc.declare_dram_parameter("bvec", [D], FP32, False)
    gamma = nc.declare_dram_parameter("gamma", [D], FP32, False)
    beta = nc.declare_dram_parameter("beta", [D], FP32, False)
    out = nc.declare_dram_parameter("out", [K, D], FP32, True)

    with ExitStack() as ctx:
        tc = ctx.enter_context(tile.TileContext(nc))

        own_pool = ctx.enter_context(tc.tile_pool(name="own", bufs=1))
        w_pool = ctx.enter_context(tc.tile_pool(name="w", bufs=1))
        wsum_pool = ctx.enter_context(tc.tile_pool(name="wsum", bufs=1))
        # Pool-recycled whole-h-tile buffers: consumer-pull pacing keeps the
        # DMA semaphore lanes unambiguous.
        emb_pool = ctx.enter_context(tc.tile_pool(name="emb", bufs=16))
        ones_pool = ctx.enter_context(tc.tile_pool(name="ones", bufs=2))
        cnt_pool = ctx.enter_context(tc.tile_pool(name="cnt", bufs=4))
        st_pool = ctx.enter_context(tc.tile_pool(name="st", bufs=3))
        bc_pool = ctx.enter_context(tc.tile_pool(name="bc", bufs=1))
        x_pool = ctx.enter_context(tc.tile_pool(name="x", bufs=3))
        stats_pool = ctx.enter_context(tc.tile_pool(name="stats", bufs=1))
        # distinct slots for the many small [K,1] epilogue vectors
        mv_pool = ctx.enter_context(tc.tile_pool(name="mv", bufs=10))

        psum_sum = ctx.enter_context(tc.tile_pool(name="psum_sum", bufs=2, space="PSUM"))
        psum_proj = ctx.enter_context(tc.tile_pool(name="psum_proj", bufs=1, space="PSUM"))
        psum_cnt = ctx.enter_context(tc.tile_pool(name="psum_cnt", bufs=1, space="PSUM"))
        psum_mean = ctx.enter_context(tc.tile_pool(name="psum_mean", bufs=1, space="PSUM"))

        def body():
            # own + wsum ride the SWDGE (gpsimd) queue: its desc-gen runs
            # off the shared HWDGE sequencer, so the emb stream's first
            # HWDGE descriptor (and the whole work-conserving pipe) starts
            # ~625ns earlier than if own led a ring.
            own_sb = own_pool.tile([P, SC, K], F8)
            nc.gpsimd.dma_start(out=own_sb[:], in_=ownP[:, :])
            wsum_sb = wsum_pool.tile([P, HC], BF)
            nc.gpsimd.dma_start(out=wsum_sb[:], in_=wsumP[:, :])

            # W quarter 0 on the scalar ring up-front; quarters 1..3
            # interleave with the emb stream.
            w_sb = w_pool.tile([P, HC, D], F8)
            nc.scalar.dma_start(w_sb[:, 0:4, :], wP[:, 0:4 * D])

            ones = ones_pool.tile([P, 1], F8)
            nc.vector.memset(ones[:], 1.0)

            # Dummy Sqrt: the act-table pass greedily picks the first table
            # containing the FIRST activation func seen; Sqrt's first match
            # (set 3, sqrt_and_others) also contains Square and Identity, so
            # this pins ONE table load, hoisted to kernel start and hidden
            # under the emb stream. Without it, Square first-matches set 0
            # and the Sqrt forces a 1283ns mid-epilogue table swap.
            dum = ones_pool.tile([1, 1], FP32)
            nc.vector.memset(dum[:], 1.0)
            nc.scalar.activation(out=dum[:], in_=dum[:],
                                 func=mybir.ActivationFunctionType.Sqrt,
                                 bias=0.0, scale=1.0, alpha=0.0)

            def bcast(vec):
                t = bc_pool.tile([K, D], FP32, name=f"bc_{vec.name}")
                ap = vec[:]
                bc_ap = bass.AP(tensor=ap.tensor, offset=ap.offset, ap=[[0, K]] + list(ap.ap))
                nc.gpsimd.dma_start(out=t[:], in_=bc_ap)
                return t

            bias_bc = bcast(bvec) if has_bias else None
            gam_bc = bcast(gamma) if has_gamma else None
            bet_bc = bcast(beta) if has_beta else None

            cnt_ps = psum_cnt.tile([K, 1], FP32)
            mean_ps = psum_mean.tile([K, 1], FP32)
            proj_ps = [psum_proj.tile([K, 512], FP32, name=f"proj_ps{jj}") for jj in range(DJ)]
            pipe = []  # (h, st_sb) awaiting their proj matmuls

            def proj_step(hh, st, stop):
                # mean first: its stop fires ~2 matmuls before the banks',
                # giving the DVE time to fold mu into bias_k during proj.
                nc.tensor.matmul(mean_ps[:], st[:], wsum_sb[:, hh:hh + 1],
                                 start=(hh == 0), stop=stop)
                for jj in range(DJ):
                    nc.tensor.matmul(proj_ps[jj][:], st[:],
                                     w_sb[:, hh, jj * 512:(jj + 1) * 512],
                                     start=(hh == 0), stop=stop)

            for h in range(HC):
                # Each h-tile streams as two halves on the two HWDGE rings
                # (SP via nc.sync, Activation ring via nc.scalar). The LAST
                # FOUR pieces (etB14, etA15, etB15x, etB15y) all ride ring A:
                # a single ring's pieces arrive strictly in issue order, so
                # the pooling emission order is guaranteed to match data
                # arrival at the stream tail (cross-ring arbitration was
                # measured inverting etB14/etA15, stalling the in-order PE
                # queue ~700ns). Tile 14's A-half compensates on ring B. The
                # final piece is only 4 chunks so just 52ns of pooling trails
                # the last DMA's 900ns completion-semaphore propagation.
                base = h * SC * P
                last = h == HC - 1
                engA = nc.scalar if h == HC - 2 else nc.sync
                etA = emb_pool.tile([P, HB, P], F8)
                engA.dma_start(etA[:], embP[:, base:base + HB * P])
                # Tile 15's B half goes on ring A right behind its A half:
                # same-ring FIFO guarantees A lands first, so pool(15)'s
                # first 16 chunks run during the final DMA and only the B
                # half's 16 chunks (208ns) trail the last 900ns semaphore.
                engB = nc.sync if h >= HC - 2 else nc.scalar
                etB = emb_pool.tile([P, HB, P], F8)
                engB.dma_start(etB[:], embP[:, base + HB * P:base + SC * P])
                if h in (2, 5, 8):
                    q = {2: 1, 5: 2, 8: 3}[h]
                    weng = nc.scalar if q == 2 else nc.sync
                    weng.dma_start(w_sb[:, 4 * q:4 * (q + 1), :],
                                   wP[:, 4 * q * D:4 * (q + 1) * D])

                st_ps = psum_sum.tile([P, 512], FP32)
                for c in range(HB):
                    nc.tensor.matmul(st_ps[:, 0:K], etA[:, c, :], own_sb[:, c, :],
                                     start=(c == 0), stop=False)
                for c in range(HB, SC):
                    nc.tensor.matmul(st_ps[:, 0:K], etB[:, c - HB, :], own_sb[:, c, :],
                                     start=False, stop=(c == SC - 1))
                if h == 0:
                    # counts[k] = sum_s own[k, s] — fills PE slack.
                    for c in range(SC):
                        nc.tensor.matmul(cnt_ps[:], own_sb[:, c, :], ones[:],
                                         start=(c == 0), stop=(c == SC - 1))
                # Stage copy on the Vector engine (Scalar queue stays pure
                # DMA issues mid-stream).
                st_sb = st_pool.tile([P, K], BF)
                nc.vector.tensor_copy(out=st_sb[:], in_=st_ps[:, 0:K])
                pipe.append((h, st_sb))
                if len(pipe) > 1 and not last:
                    hh, st = pipe.pop(0)
                    proj_step(hh, st, stop=False)
            # pipe holds (14, 15): proj(14) lands right after pool(15)'s
            # last chunk (its operands are long ready) and overlaps the DVE
            # stage copy of tile 15; proj(15) follows.
            hh, st = pipe.pop(0)
            proj_step(hh, st, stop=False)
            hh, st = pipe.pop(0)
            proj_step(hh, st, stop=True)

            cnt_sb = cnt_pool.tile([K, 1], FP32)
            nc.vector.tensor_scalar_max(out=cnt_sb[:], in0=cnt_ps[:], scalar1=1.0)

            if has_bias:
                # General path: pooled = summed/counts materialized before
                # the bias add; plain-eps LayerNorm via bn_stats.
                cnt64 = cnt_pool.tile([K, 1], FP32)
                nc.vector.tensor_scalar_mul(out=cnt64[:], in0=cnt_sb[:], scalar1=W_SCALE)
                inv_sb = cnt_pool.tile([K, 1], FP32)
                nc.vector.reciprocal(out=inv_sb[:], in_=cnt64[:])
                eps_k = cnt_pool.tile([K, 1], FP32)
                nc.vector.memset(eps_k[:], LN_EPS)
                x = x_pool.tile([K, D], FP32)
                for jj in range(DJ):
                    nc.vector.tensor_scalar_mul(
                        out=x[:, jj * 512:(jj + 1) * 512], in0=proj_ps[jj][:], scalar1=inv_sb[:],
                    )
                nc.vector.tensor_add(out=x[:], in0=x[:], in1=bias_bc[:])
                src = [x[:, jj * 512:(jj + 1) * 512] for jj in range(DJ)]

                stats = stats_pool.tile([K, DJ, nc.vector.BN_STATS_DIM], FP32)
                for g in range(DJ):
                    nc.vector.bn_stats(out=stats[:, g, :], in_=src[g])
                mv = mv_pool.tile([K, nc.vector.BN_AGGR_DIM], FP32)
                nc.vector.bn_aggr(out=mv[:], in_=stats[:])
                rstd = mv_pool.tile([K, 1], FP32)
                nc.scalar.activation(
                    out=rstd[:], in_=mv[:, 1:2],
                    func=mybir.ActivationFunctionType.Sqrt, bias=eps_k[:], scale=1.0, alpha=0.0,
                )
                nc.vector.reciprocal(out=rstd[:], in_=rstd[:])
                outt = x_pool.tile([K, D], FP32)
                for jj in range(DJ):
                    half = outt[:, jj * 512:(jj + 1) * 512]
                    nc.vector.tensor_scalar(
                        out=half, in0=src[jj], scalar1=mv[:, 0:1], scalar2=rstd[:],
                        op0=mybir.AluOpType.subtract, op1=mybir.AluOpType.mult,
                    )
                    if has_gamma:
                        nc.vector.tensor_mul(out=half, in0=half, in1=gam_bc[:, jj * 512:(jj + 1) * 512])
                    if has_beta:
                        nc.vector.tensor_add(out=half, in0=half, in1=bet_bc[:, jj * 512:(jj + 1) * 512])
                    oeng = nc.sync if jj == 0 else nc.scalar
                    oeng.dma_start(out[:, jj * 512:(jj + 1) * 512], half)
                return

            # Fast path (bias==0, gamma==1, beta==0): LN scale invariance
            # normalizes the raw v = W_SCALE*summed@W directly.
            #   mu      = mean_ps / D          (mean_ps = sum_d v, from PE)
            #   var     = sumsq/D - mu^2
            #   rstd    = 1/sqrt(var + eps_k),  eps_k = LN_EPS*(c*W_SCALE)^2
            #   out     = (v - mu) * rstd
            cnt2 = cnt_pool.tile([K, 1], FP32)
            nc.vector.tensor_mul(out=cnt2[:], in0=cnt_sb[:], in1=cnt_sb[:])
            eps_k = cnt_pool.tile([K, 1], FP32)
            nc.vector.tensor_scalar_mul(out=eps_k[:], in0=cnt2[:],
                                        scalar1=LN_EPS * W_SCALE * W_SCALE)

            mu = mv_pool.tile([K, 1], FP32)
            nc.vector.tensor_scalar_mul(out=mu[:], in0=mean_ps[:], scalar1=1.0 / D)
            musq = mv_pool.tile([K, 1], FP32)
            nc.vector.tensor_mul(out=musq[:], in0=mu[:], in1=mu[:])
            # bias_k = eps_k - mu^2, folded into the Sqrt's bias so the
            # post-sumsq chain is just add -> sqrt -> recip -> negmurstd.
            bias_k = mv_pool.tile([K, 1], FP32)
            nc.vector.tensor_sub(out=bias_k[:], in0=eps_k[:], in1=musq[:])

            # bank0's accumulation stops ~213ns before bank1's, so the ACT
            # engine (slower leg: 612 + 187 accumulator read) takes bank0
            # and the DVE takes bank1 — both legs finish together. The DVE
            # leg uses bn_stats (a DVE op may read only ONE input from PSUM,
            # which rules out tensor_tensor_reduce(x,x)); bank1's sum of
            # squares is recovered as 512*(var1 + mean1^2).
            sumsq0 = mv_pool.tile([K, 1], FP32)
            scr0 = x_pool.tile([K, 512], FP32)
            nc.scalar.activation(
                out=scr0[:], in_=proj_ps[0][:],
                func=mybir.ActivationFunctionType.Square,
                bias=0.0, scale=1.0, alpha=0.0, accum_out=sumsq0[:],
            )
            stats1 = stats_pool.tile([K, 1, nc.vector.BN_STATS_DIM], FP32)
            nc.vector.bn_stats(out=stats1[:, 0, :], in_=proj_ps[1][:])
            mv1 = mv_pool.tile([K, nc.vector.BN_AGGR_DIM], FP32)
            nc.vector.bn_aggr(out=mv1[:], in_=stats1[:])
            m1sq = mv_pool.tile([K, 1], FP32)
            nc.vector.tensor_mul(out=m1sq[:], in0=mv1[:, 0:1], in1=mv1[:, 0:1])
            h1 = mv_pool.tile([K, 1], FP32)
            nc.vector.tensor_add(out=h1[:], in0=mv1[:, 1:2], in1=m1sq[:])
            # t = 0.5*h1 + (eps_k - mu^2); full sqrt arg = sumsq0/D + t is
            # folded into the Sqrt's scale/bias (sumsq0 is ACT-side SBUF).
            tk = mv_pool.tile([K, 1], FP32)
            nc.vector.tensor_scalar(
                out=tk[:], in0=h1[:], scalar1=0.5, scalar2=bias_k[:],
                op0=mybir.AluOpType.mult, op1=mybir.AluOpType.add,
            )
            rstd = mv_pool.tile([K, 1], FP32)
            nc.scalar.activation(
                out=rstd[:], in_=sumsq0[:],
                func=mybir.ActivationFunctionType.Sqrt,
                bias=tk[:], scale=1.0 / D, alpha=0.0,
            )
            nc.vector.reciprocal(out=rstd[:], in_=rstd[:])
            negmurstd = mv_pool.tile([K, 1], FP32)
            nc.vector.tensor_scalar(
                out=negmurstd[:], in0=mu[:], scalar1=rstd[:], scalar2=-1.0,
                op0=mybir.AluOpType.mult, op1=mybir.AluOpType.mult,
            )
            outt = x_pool.tile([K, D], FP32)
            nc.vector.tensor_scalar(
                out=outt[:, 0:512], in0=proj_ps[0][:], scalar1=mu[:], scalar2=rstd[:],
                op0=mybir.AluOpType.subtract, op1=mybir.AluOpType.mult,
            )
            nc.scalar.activation(
                out=outt[:, 512:1024], in_=proj_ps[1][:],
                func=mybir.ActivationFunctionType.Identity,
                bias=negmurstd[:], scale=rstd[:], alpha=0.0,
            )
            nc.sync.dma_start(out[:, :], outt[:])

        for _ in range(repeats):
            body()

    nc.finalize()
    return nc


def kernel(**inputs: np.ndarray) -> np.ndarray:
    global _NC, _NC_KEY, LAST_RESULT
    from concourse.bass_utils import run_bass_kernel_spmd

    emb = np.asarray(inputs["plan_embeddings"], dtype=np.float32)
    own = np.asarray(inputs["ownership"])
    wmat = np.ascontiguousarray(np.asarray(inputs["W"], dtype=np.float32))
    bv = np.ascontiguousarray(np.asarray(inputs["b"], dtype=np.float32))
    ga = np.ascontiguousarray(np.asarray(inputs["gamma"], dtype=np.float32))
    be = np.ascontiguousarray(np.asarray(inputs["beta"], dtype=np.float32))

    key = (bool(np.any(bv != 0.0)), bool(np.any(ga != 1.0)), bool(np.any(be != 0.0)))
    if _NC is None or _NC_KEY != key:
        _NC = _build_nc(has_bias=key[0], has_gamma=key[1], has_beta=key[2])
        _NC_KEY = key

    wP = _prep_w(wmat)
    wsumP = _prep_wsum(wmat)
    in_maps = []
    for i in range(B):
        in_maps.append({
            "embP": _prep_emb(emb[i]),
            "ownP": _prep_own(own[i]),
            "wP": wP,
            "wsumP": wsumP,
            "bvec": bv,
            "gamma": ga,
            "beta": be,
        })
    res = run_bass_kernel_spmd(_NC, in_maps, core_ids=list(range(B)), trace=TRACE,
                               tmpdir=TRACE_TMPDIR)
    LAST_RESULT = res
    return np.stack([np.asarray(res.results[i]["out"]) for i in range(B)], axis=0).astype(np.float32)


# revision 4
# speedup vs baseline: 3.2138x; 1.0991x over previous
"""Trainium2 Bass kernel for nn_PlanNotesProjection — v2 (tail-optimized).

Math (per batch b):
  own_f   = ownership[b].astype(f32)             # (K=32, S=4096)
  summed  = own_f @ emb[b]                       # (K, H=2048)
  counts  = clip(own_f.sum(-1), min=1)           # (K,)
  pooled  = summed / counts[:, None]
  proj    = pooled @ W + bias                    # (K, D=1024)
  out[b]  = LayerNorm(proj) * gamma + beta       # eps=1e-5

v2 changes vs v1 (all aimed at the post-stream tail, which the timeline
sim showed was 8.25us of the 39.7us total):
  1. Host-centered W: wP holds W' = W - rowmean(W), so every projection
     row has exactly zero mean by construction (sum_d v'_kd = pooled_k @
     rowsums(W') = 0; the fp8 re-quantization leaves a residual mean of
     ~0.04% of sigma_v). The LN epilogue needs no mean at all — just a
     sum of squares and a per-row scale.
  2. Parallel sumsq: bank0 via ACT activation(Square, accum_out);
     bank1 via DVE bn_stats (a DVE op may read only one PSUM input, so
     sumsq1 = 512*(var1+mean1^2)) — the two legs run concurrently.
  3. Parallel normalize (pure per-row scale): bank0 via DVE
     tensor_scalar_mul(rstd); bank1 via ACT activation(Identity,
     scale=rstd). Square/Sqrt/Identity all live in one ACT table
     (sqrt_and_others, pinned by an early dummy Sqrt) so no mid-kernel
     ACT table swap.
  4. Single [K,1024] store (one 625ns HWDGE desc-gen + one 650ns DGE
     launch instead of two serialized chains).
  5. Tail scheduling: proj(14) and proj(15) are emitted after pool(15)
     so proj(14) (whose operands are long ready) fills the PE while the
     DVE stage-copy of tile 15 round-trips; tile 14's halves swap rings
     so the slower Act-ring SEQ doesn't reorder the last arrivals.

Everything else (fp8 e3m4 streaming, host swizzles, two HWDGE rings,
depth-2 proj software pipeline, LN scale-invariance absorbing the
counts division and W_SCALE) is unchanged from v1 — see below.

Sharding: data-parallel over B across 8 cores (one batch per core);
W/b/gamma/beta replicated. Host swizzles make every device DMA fully
contiguous per partition:
  embP[p, (h*SC + c)*128 + j] = emb[c*128+p, h*128+j]   (fp8 e3m4)
  ownP[p, c*K + k]            = own[k, c*128+p]          (fp8 e3m4)
  wP[p, h*D + d]              = 64 * (W - rowmean(W))[h*128+p, d]  (fp8)
"""

import sys
from contextlib import ExitStack

import numpy as np

sys.path.insert(0, "/opt/trn_rl_repo")

import ml_dtypes

BF16 = ml_dtypes.bfloat16
FP8E3 = ml_dtypes.float8_e3m4

B, K, S, H, D = 8, 32, 4096, 2048, 1024
LN_EPS = 1e-5
P = 128
SC = S // P    # 32 contraction chunks (S on partitions)
HC = H // P    # 16 h-tiles
DJ = 2        # psum column tiles for projection
BW = D // DJ  # 512 columns per projection bank
HB = SC // 2   # half an h-tile's chunks per DMA
LB = 4         # trailing chunks in the split final DMA

TRACE = False
TRACE_TMPDIR = None
LAST_RESULT = None
_NC = None
_NC_KEY = None


def _prep_emb(emb_b):
    # (S, H) f32 -> (P, HC*SC*128) fp8 e3m4 with
    # embP[p, (h*SC+c)*128+j] = emb[c*128+p, h*128+j]
    return np.ascontiguousarray(
        emb_b.astype(FP8E3).reshape(SC, P, HC, P).transpose(1, 2, 0, 3)
        .reshape(P, HC * SC * P))


def _prep_own(own_b):
    # (K, S) bool -> (P, SC*K) fp8 with ownP[p, c*K+k] = own[k, c*128+p]
    return np.ascontiguousarray(
        own_b.T.astype(FP8E3).reshape(SC, P, K).transpose(1, 0, 2).reshape(P, SC * K))


W_SCALE = 64.0  # lifts W ~N(0, 1/sqrt(2048)) into e3m4's normal range


def _prep_w(wmat):
    # (H, D) f32 -> (P, HC*D) fp8 e3m4. W is row-CENTERED on the host
    # (W' = W - rowmean(W)) before scaling: every projection row then has
    # exactly zero mean by construction (sum_d v'_kd = pooled_k @ rowsums
    # = 0), so the LayerNorm epilogue needs no mean at all — only the sum
    # of squares. The fp8 quantization of W' leaves a residual row-sum of
    # ~0.04% of sigma_v: far below the error budget. The global W_SCALE is
    # absorbed by LayerNorm scale invariance (eps_k picks up W_SCALE^2).
    wc = wmat - wmat.mean(axis=1, keepdims=True)
    return np.ascontiguousarray(
        (wc * W_SCALE).astype(FP8E3).reshape(HC, P, D).transpose(1, 0, 2)
        .reshape(P, HC * D))


def _build_nc(repeats=1, has_bias=False, has_gamma=False, has_beta=False):
    import concourse.bass as bass
    import concourse.tile as tile
    from concourse import mybir
    from concourse.bacc import Bacc

    FP32 = mybir.dt.float32
    BF = mybir.dt.bfloat16
    F8 = mybir.dt.float8e3

    nc = Bacc("TRN2", target_bir_lowering=False)
    embP = nc.declare_dram_parameter("embP", [P, HC * SC * P], F8, False)
    ownP = nc.declare_dram_parameter("ownP", [P, SC * K], F8, False)
    wP = nc.declare_dram_parameter("wP", [P, HC * D], F8, False)
    bvec = nc.declare_dram_parameter("bvec", [D], FP32, False)
    gamma = nc.declare_dram_parameter("gamma", [D], FP32, False)
    beta = nc.declare_dram_parameter("beta", [D], FP32, False)
    out = nc.declare_dram_parameter("out", [K, D], FP32, True)

    with ExitStack() as ctx:
        tc = ctx.enter_context(tile.TileContext(nc))

        own_pool = ctx.enter_context(tc.tile_pool(name="own", bufs=1))
        w_pool = ctx.enter_context(tc.tile_pool(name="w", bufs=1))
        # Pool-recycled whole-h-tile buffers: consumer-pull pacing keeps the
        # DMA semaphore lanes unambiguous.
        emb_pool = ctx.enter_context(tc.tile_pool(name="emb", bufs=16))
        ones_pool = ctx.enter_context(tc.tile_pool(name="ones", bufs=2))
        cnt_pool = ctx.enter_context(tc.tile_pool(name="cnt", bufs=4))
        st_pool = ctx.enter_context(tc.tile_pool(name="st", bufs=3))
        bc_pool = ctx.enter_context(tc.tile_pool(name="bc", bufs=1))
        x_pool = ctx.enter_context(tc.tile_pool(name="x", bufs=3))
        stats_pool = ctx.enter_context(tc.tile_pool(name="stats", bufs=1))
        # distinct slots for the many small [K,1] epilogue vectors
        mv_pool = ctx.enter_context(tc.tile_pool(name="mv", bufs=12))

        psum_sum = ctx.enter_context(tc.tile_pool(name="psum_sum", bufs=2, space="PSUM"))
        psum_proj = ctx.enter_context(tc.tile_pool(name="psum_proj", bufs=1, space="PSUM"))
        psum_cnt = ctx.enter_context(tc.tile_pool(name="psum_cnt", bufs=1, space="PSUM"))

        def body():
            # own + wsum ride the SWDGE (gpsimd) queue: its desc-gen runs
            # off the shared HWDGE sequencer, so the emb stream's first
            # HWDGE descriptor (and the whole work-conserving pipe) starts
            # ~625ns earlier than if own led a ring.
            own_sb = own_pool.tile([P, SC, K], F8)
            nc.gpsimd.dma_start(out=own_sb[:], in_=ownP[:, :])

            # W quarter 0 on the scalar ring up-front; quarters 1..3
            # interleave with the emb stream.
            w_sb = w_pool.tile([P, HC, D], F8)
            nc.scalar.dma_start(w_sb[:, 0:4, :], wP[:, 0:4 * D])

            ones = ones_pool.tile([P, 1], F8)
            nc.vector.memset(ones[:], 1.0)

            # Dummy Sqrt: the act-table pass greedily picks the first table
            # containing the FIRST activation func seen; Sqrt's first match
            # (set 3, sqrt_and_others) also contains Square and Identity, so
            # this pins ONE table load, hoisted to kernel start and hidden
            # under the emb stream. Without it, Square first-matches set 0
            # and the Sqrt forces a 1283ns mid-epilogue table swap.
            dum = ones_pool.tile([1, 1], FP32)
            nc.vector.memset(dum[:], 1.0)
            nc.scalar.activation(out=dum[:], in_=dum[:],
                                 func=mybir.ActivationFunctionType.Sqrt,
                                 bias=0.0, scale=1.0, alpha=0.0)

            def bcast(vec):
                t = bc_pool.tile([K, D], FP32, name=f"bc_{vec.name}")
                ap = vec[:]
                bc_ap = bass.AP(tensor=ap.tensor, offset=ap.offset, ap=[[0, K]] + list(ap.ap))
                nc.gpsimd.dma_start(out=t[:], in_=bc_ap)
                return t

            bias_bc = bcast(bvec) if has_bias else None
            gam_bc = bcast(gamma) if has_gamma else None
            bet_bc = bcast(beta) if has_beta else None

            cnt_ps = psum_cnt.tile([K, 1], FP32)
            proj_ps = [psum_proj.tile([K, BW], FP32, name=f"proj_ps{jj}") for jj in range(DJ)]
            pipe = []  # (h, st_sb) awaiting their proj matmuls

            def proj_step(hh, st, stop):
                for jj in range(DJ):
                    nc.tensor.matmul(proj_ps[jj][:], st[:],
                                     w_sb[:, hh, jj * BW:(jj + 1) * BW],
                                     start=(hh == 0), stop=stop)

            for h in range(HC):
                # Each h-tile streams as two halves on the two HWDGE rings
                # (SP via nc.sync, Activation ring via nc.scalar). The LAST
                # FOUR pieces (etB14, etA15, etB15x, etB15y) all ride ring A:
                # a single ring's pieces arrive strictly in issue order, so
                # the pooling emission order is guaranteed to match data
                # arrival at the stream tail (cross-ring arbitration was
                # measured inverting etB14/etA15, stalling the in-order PE
                # queue ~700ns). Tile 14's A-half compensates on ring B. The
                # final piece is only 4 chunks so just 52ns of pooling trails
                # the last DMA's 900ns completion-semaphore propagation.
                base = h * SC * P
                last = h == HC - 1
                engA = nc.scalar if h == HC - 2 else nc.sync
                etA = emb_pool.tile([P, HB, P], F8)
                if last:
                    with tc.high_priority():
                        engA.dma_start(etA[:], embP[:, base:base + HB * P])
                else:
                    engA.dma_start(etA[:], embP[:, base:base + HB * P])
                # Tile 15's B half goes on ring A right behind its A half:
                # same-ring FIFO guarantees A lands first, so pool(15)'s
                # first 16 chunks run during the final DMA and only the B
                # half's 16 chunks (208ns) trail the last 900ns semaphore.
                engB = nc.sync if h >= HC - 2 else nc.scalar
                etB = emb_pool.tile([P, HB, P], F8)
                if last:
                    with tc.high_priority():
                        engB.dma_start(etB[:], embP[:, base + HB * P:base + SC * P])
                else:
                    engB.dma_start(etB[:], embP[:, base + HB * P:base + SC * P])
                if h in (2, 5, 8):
                    q = {2: 1, 5: 2, 8: 3}[h]
                    weng = nc.scalar if q == 2 else nc.sync
                    weng.dma_start(w_sb[:, 4 * q:4 * (q + 1), :],
                                   wP[:, 4 * q * D:4 * (q + 1) * D])

                st_ps = psum_sum.tile([P, 512], FP32)
                for c in range(HB):
                    nc.tensor.matmul(st_ps[:, 0:K], etA[:, c, :], own_sb[:, c, :],
                                     start=(c == 0), stop=False)
                for c in range(HB, SC):
                    nc.tensor.matmul(st_ps[:, 0:K], etB[:, c - HB, :], own_sb[:, c, :],
                                     start=False, stop=(c == SC - 1))
                if h == 0:
                    # counts[k] = sum_s own[k, s] — fills PE slack.
                    for c in range(SC):
                        nc.tensor.matmul(cnt_ps[:], own_sb[:, c, :], ones[:],
                                         start=(c == 0), stop=(c == SC - 1))
                # Stage copy on the Vector engine (Scalar queue stays pure
                # DMA issues mid-stream).
                st_sb = st_pool.tile([P, K], BF)
                nc.vector.tensor_copy(out=st_sb[:], in_=st_ps[:, 0:K])
                pipe.append((h, st_sb))
                if len(pipe) > 1 and not last:
                    hh, st = pipe.pop(0)
                    proj_step(hh, st, stop=False)
            # pipe holds (14, 15): proj(14) lands right after pool(15)'s
            # last chunk (its operands are long ready) and overlaps the DVE
            # stage copy of tile 15; proj(15) follows.
            hh, st = pipe.pop(0)
            proj_step(hh, st, stop=False)
            hh, st = pipe.pop(0)
            proj_step(hh, st, stop=True)

            cnt_sb = cnt_pool.tile([K, 1], FP32)
            nc.vector.tensor_scalar_max(out=cnt_sb[:], in0=cnt_ps[:], scalar1=1.0)

            if has_bias:
                # General path: pooled = summed/counts materialized before
                # the bias add; plain-eps LayerNorm via bn_stats.
                cnt64 = cnt_pool.tile([K, 1], FP32)
                nc.vector.tensor_scalar_mul(out=cnt64[:], in0=cnt_sb[:], scalar1=W_SCALE)
                inv_sb = cnt_pool.tile([K, 1], FP32)
                nc.vector.reciprocal(out=inv_sb[:], in_=cnt64[:])
                eps_k = cnt_pool.tile([K, 1], FP32)
                nc.vector.memset(eps_k[:], LN_EPS)
                x = x_pool.tile([K, D], FP32)
                for jj in range(DJ):
                    nc.vector.tensor_scalar_mul(
                        out=x[:, jj * BW:(jj + 1) * BW], in0=proj_ps[jj][:], scalar1=inv_sb[:],
                    )
                nc.vector.tensor_add(out=x[:], in0=x[:], in1=bias_bc[:])
                src = [x[:, jj * BW:(jj + 1) * BW] for jj in range(DJ)]

                stats = stats_pool.tile([K, DJ, nc.vector.BN_STATS_DIM], FP32)
                for g in range(DJ):
                    nc.vector.bn_stats(out=stats[:, g, :], in_=src[g])
                mv = mv_pool.tile([K, nc.vector.BN_AGGR_DIM], FP32)
                nc.vector.bn_aggr(out=mv[:], in_=stats[:])
                rstd = mv_pool.tile([K, 1], FP32)
                nc.scalar.activation(
                    out=rstd[:], in_=mv[:, 1:2],
                    func=mybir.ActivationFunctionType.Sqrt, bias=eps_k[:], scale=1.0, alpha=0.0,
                )
                nc.vector.reciprocal(out=rstd[:], in_=rstd[:])
                outt = x_pool.tile([K, D], FP32)
                for jj in range(DJ):
                    half = outt[:, jj * BW:(jj + 1) * BW]
                    nc.vector.tensor_scalar(
                        out=half, in0=src[jj], scalar1=mv[:, 0:1], scalar2=rstd[:],
                        op0=mybir.AluOpType.subtract, op1=mybir.AluOpType.mult,
                    )
                    if has_gamma:
                        nc.vector.tensor_mul(out=half, in0=half, in1=gam_bc[:, jj * BW:(jj + 1) * BW])
                    if has_beta:
                        nc.vector.tensor_add(out=half, in0=half, in1=bet_bc[:, jj * BW:(jj + 1) * BW])
                nc.sync.dma_start(out[:, :], outt[:])
                return

            # Fast path (bias==0, gamma==1, beta==0): LN scale invariance
            # normalizes the raw v = W_SCALE*summed@Wc directly, and the
            # host-centered W makes every row of v exactly zero-mean, so
            # the epilogue is just a sum-of-squares and a scale:
            #   rstd = 1/sqrt(sumsq/D + eps_k),  eps_k = LN_EPS*(c*W_SCALE)^2
            #   out  = v * rstd
            cnt2 = cnt_pool.tile([K, 1], FP32)
            nc.vector.tensor_mul(out=cnt2[:], in0=cnt_sb[:], in1=cnt_sb[:])
            eps_k = cnt_pool.tile([K, 1], FP32)
            nc.vector.tensor_scalar_mul(out=eps_k[:], in0=cnt2[:],
                                        scalar1=LN_EPS * W_SCALE * W_SCALE)

            # Stats split across engines and banks (ACT and DVE may touch
            # PSUM concurrently only on DIFFERENT banks): ACT Square-
            # accumulates bank0 (the slower leg, started first) while DVE
            # bn_stats bank1 (a DVE op may read only ONE input from PSUM,
            # ruling out tensor_tensor_reduce(x,x)); bank1's sum of squares
            # is recovered as 512*(var1 + mean1^2).
            sumsq0 = mv_pool.tile([K, 1], FP32)
            scr0 = x_pool.tile([K, BW], FP32)
            nc.scalar.activation(
                out=scr0[:], in_=proj_ps[0][:],
                func=mybir.ActivationFunctionType.Square,
                bias=0.0, scale=1.0, alpha=0.0, accum_out=sumsq0[:],
            )
            stats1 = stats_pool.tile([K, 1, nc.vector.BN_STATS_DIM], FP32)
            nc.vector.bn_stats(out=stats1[:, 0, :], in_=proj_ps[1][:])
            mv1 = mv_pool.tile([K, nc.vector.BN_AGGR_DIM], FP32)
            nc.vector.bn_aggr(out=mv1[:], in_=stats1[:])
            m1sq = mv_pool.tile([K, 1], FP32)
            nc.vector.tensor_mul(out=m1sq[:], in0=mv1[:, 0:1], in1=mv1[:, 0:1])
            h1 = mv_pool.tile([K, 1], FP32)
            nc.vector.tensor_add(out=h1[:], in0=mv1[:, 1:2], in1=m1sq[:])
            # tk = 0.5*h1 + eps_k; the ACT-side sum of squares is folded
            # into the Sqrt as in*scale: rstd_pre = Sqrt(sumsq0/D + tk).
            tk = mv_pool.tile([K, 1], FP32)
            nc.vector.tensor_scalar(
                out=tk[:], in0=h1[:], scalar1=0.5, scalar2=eps_k[:],
                op0=mybir.AluOpType.mult, op1=mybir.AluOpType.add,
            )
            rstd = mv_pool.tile([K, 1], FP32)
            nc.scalar.activation(
                out=rstd[:], in_=sumsq0[:],
                func=mybir.ActivationFunctionType.Sqrt,
                bias=tk[:], scale=1.0 / D, alpha=0.0,
            )
            nc.vector.reciprocal(out=rstd[:], in_=rstd[:])
            # Normalize is a pure per-row scale: DVE bank0, ACT bank1 on
            # distinct PSUM banks concurrently.
            outt = x_pool.tile([K, D], FP32)
            nc.vector.tensor_scalar_mul(
                out=outt[:, 0:BW], in0=proj_ps[0][:], scalar1=rstd[:],
            )
            nc.scalar.activation(
                out=outt[:, BW:2 * BW], in_=proj_ps[1][:],
                func=mybir.ActivationFunctionType.Identity,
                bias=0.0, scale=rstd[:], alpha=0.0,
            )
            nc.sync.dma_start(out[:, :], outt[:])

        for _ in range(repeats):
            body()

    nc.finalize()
    return nc


def kernel(**inputs: np.ndarray) -> np.ndarray:
    global _NC, _NC_KEY, LAST_RESULT
    from concourse.bass_utils import run_bass_kernel_spmd

    emb = np.asarray(inputs["plan_embeddings"], dtype=np.float32)
    own = np.asarray(inputs["ownership"])
    wmat = np.ascontiguousarray(np.asarray(inputs["W"], dtype=np.float32))
    bv = np.ascontiguousarray(np.asarray(inputs["b"], dtype=np.float32))
    ga = np.ascontiguousarray(np.asarray(inputs["gamma"], dtype=np.float32))
    be = np.ascontiguousarray(np.asarray(inputs["beta"], dtype=np.float32))

    key = (bool(np.any(bv != 0.0)), bool(np.any(ga != 1.0)), bool(np.any(be != 0.0)))
    if _NC is None or _NC_KEY != key:
        _NC = _build_nc(has_bias=key[0], has_gamma=key[1], has_beta=key[2])
        _NC_KEY = key

    wP = _prep_w(wmat)
    in_maps = []
    for i in range(B):
        in_maps.append({
            "embP": _prep_emb(emb[i]),
            "ownP": _prep_own(own[i]),
            "wP": wP,
            "bvec": bv,
            "gamma": ga,
            "beta": be,
        })
    res = run_bass_kernel_spmd(_NC, in_maps, core_ids=list(range(B)), trace=TRACE,
                               tmpdir=TRACE_TMPDIR)
    LAST_RESULT = res
    return np.stack([np.asarray(res.results[i]["out"]) for i in range(B)], axis=0).astype(np.float32)
